# revision 26
# baseline (speedup 1.0000x reference)
"""Context-aware attention pooling kernel for Trainium2 (8 NeuronCores).

Reference computation (per batch b):
    e      = tanh(seq @ W1[:256] + ctx @ W1[256:])      # [T, 64]
    logits = e @ W2                                      # [T, 1]
    a      = softmax(logits over T)
    out    = sum_t a[t] * seq[t]                         # [256]

Shapes: B=64, T=4096, D1=256, D2=128, UNITS=64.
Sharding: data-parallel over batch, 8 batches per core; W1/W2 replicated.

Host-side prep (make_in_maps) ships two copies of seq per core:
  - natp  [8, 128, 32*256] bf16: nat[b, p, n*256+d] = seq[b, 128n+p, d]
    (pool operand; t on partitions)
  - seqt  [8, 128, 2*4096] fp8e4m3: seqt[b, q, h*T+t] = seq[b, t, 128h+q]
    (e-matmul moving operand; d on partitions, pre-transposed on host so
    the PE does zero transposes; fp8 feeds only the tanh argument, the
    value path stays bf16)

Per-core program (per batch):
  - e-matmul: 4 PSUM tiles [128, 512]; even 512-chunk units on partitions
    0:64, odd on 64:128 (tile_position col split), K=256 via 2 accumulating
    matmuls vs the two seqt d-halves; tanh + ctx-bias fused on ScalarE in
    one [128, 512] activation per double-chunk
  - logits: one LDWEIGHTS (eT2 128x128 window) + one 2-column matmul per
    128 t's; rhs = [w2;0 | 0;w2] so even/odd chunk logits come out in one
    instruction pair; FWL stays enabled (no fp32 matmuls in steady state)
  - softmax without max-subtraction; single Exp over [128, 32] with fused
    row-sums; Z via ones-matmul; 1/Z applied to the pooled output
  - pooling on PE: p-columns stationary (1-col weight loads), nat tiles
    moving, accumulated over 32 t-tiles into PSUM [1, 256]
  - bf16 dummy matmuls trip the HAM clock gate during the DMA ramp
"""

import numpy as np
import ml_dtypes

import concourse.bacc as bacc
import concourse.mybir as mybir
from concourse.tile import TileContext

F32 = mybir.dt.float32
BF16 = mybir.dt.bfloat16
F8 = mybir.dt.float8e4

N_CORES = 8
B_CORE = 8          # batches per core
T = 4096
D1 = 256
D2 = 128
U = 64
NT = T // 128       # 32 t-tiles per batch

SEQT_FP8 = True     # fp8 e-path (rel err ~1.1e-2) vs bf16 (~2.5e-3)
SEQT_DT = F8 if SEQT_FP8 else BF16
SEQT_NP = ml_dtypes.float8_e4m3fn if SEQT_FP8 else ml_dtypes.bfloat16


def build_program():
    nc = bacc.Bacc("TRN2", target_bir_lowering=False, debug=False)

    natp = nc.declare_dram_parameter("natp", [B_CORE, 128, NT * D1], BF16, isOutput=False)
    seqt = nc.declare_dram_parameter("seqt", [B_CORE, 128, 2 * T], SEQT_DT, isOutput=False)
    # packed weights: one bf16 tensor (w1s | w2two) and one f32 (w1c | ctxT)
    # so the boot path is 2 SWDGE DMAs instead of 4
    wbf = nc.declare_dram_parameter("wbf", [128, 2 * U + 2], BF16, isOutput=False)
    wf32 = nc.declare_dram_parameter("wf32", [D2, U + B_CORE], F32, isOutput=False)
    outp = nc.declare_dram_parameter("outp", [1, B_CORE * D1], F32, isOutput=True)

    with TileContext(nc) as tc:
        with (
            tc.tile_pool(name="singles", bufs=1) as singles,
            tc.tile_pool(name="nat_pool", bufs=4) as nat_pool,
            tc.tile_pool(name="seqt_pool", bufs=4) as seqt_pool,
            tc.tile_pool(name="et_pool", bufs=2) as et_pool,
            tc.tile_pool(name="small_pool", bufs=2) as small_pool,
            tc.tile_pool(name="ps", bufs=1, space="PSUM") as ps,
        ):
            # Everything rides the gpsimd SWDGE queue: HWDGE descriptor
            # generation can't keep up with 128-partition APs (measured
            # ~20x starvation vs SWDGE), and a single FIFO queue delivers
            # bytes exactly in consumption order. Weights first (52 KB).
            wbf_sb = singles.tile([128, 2 * U + 2], BF16)
            nc.gpsimd.dma_start(out=wbf_sb, in_=wbf[:, :])
            wf32_sb = singles.tile([D2, U + B_CORE], F32)
            nc.gpsimd.dma_start(out=wf32_sb, in_=wf32[:, :])
            w1s_sb = wbf_sb[:, 0 : 2 * U]
            w2t = wbf_sb[:, 2 * U : 2 * U + 2]
            w1c_sb = wf32_sb[:, 0:U]
            ctxT_sb = wf32_sb[:, U : U + B_CORE]

            # seq loads: seqt (e-path) on sync/HWDGE, nat (pool) on gpsimd/SWDGE
            seqt_tiles = [None] * B_CORE
            nat_tiles = [None] * B_CORE

            def load_seqt(b, nchunks=2):
                st = seqt_pool.tile([128, 2 * T], SEQT_DT, tag="seqt", name=f"st{b}")
                st3 = st.rearrange("q (h t) -> q h t", h=2)
                src = seqt[b].rearrange("q (h t) -> q h t", h=2)
                for q in range(nchunks):
                    tsl = slice(T // nchunks * q, T // nchunks * (q + 1))
                    nc.gpsimd.dma_start(out=st3[:, :, tsl], in_=src[:, :, tsl])
                seqt_tiles[b] = st

            def load_nat(b):
                nat = nat_pool.tile([128, NT * D1], BF16, tag="nat", name=f"nat{b}")
                for q in range(4):
                    csl = slice(2048 * q, 2048 * (q + 1))
                    nc.gpsimd.dma_start(out=nat[:, csl], in_=natp[b][:, csl])
                nat_tiles[b] = nat

            ones_col = singles.tile([128, 1], F32)
            nc.vector.memset(ones_col, 1.0)

            load_seqt(0)
            load_nat(0)
            load_seqt(1)
            load_nat(1)
            load_seqt(2)
            load_nat(2)
            load_seqt(3)
            load_nat(3)

            # all 8 context projections, duplicated on both partition halves
            cb_ps = ps.tile([128, B_CORE], F32, tag="lg", bufs=2)
            nc.tensor.matmul(cb_ps[0:U], lhsT=w1c_sb, rhs=ctxT_sb, start=True, stop=True)
            nc.tensor.matmul(
                cb_ps[U:128], lhsT=w1c_sb, rhs=ctxT_sb, start=True, stop=True,
                tile_position=(0, U),
            )
            cb_all = singles.tile([128, B_CORE], F32)
            nc.scalar.copy(cb_all, cb_ps)

            # HAM warm-up: bf16 dummy matmuls during the DMA ramp so batch 0
            # computes at the full 2.4 GHz clock
            warm_ps = ps.tile([128, 128], F32, tag="warm", bufs=1)
            for _ in range(36):
                nc.tensor.matmul(warm_ps, lhsT=w1s_sb, rhs=w1s_sb, start=True, stop=True)

            final_sb = singles.tile([1, B_CORE * D1], F32)

            eT2s = [None] * B_CORE
            pABs = [None] * B_CORE

            def e_chunk(b, k):
                # eT2[:, 512k + i]: rows 0:64 = units for t = 1024k + i,
                # rows 64:128 = units for t = 1024k + 512 + i
                st3 = seqt_tiles[b].rearrange("q (h t) -> q h t", h=2)
                e_ps = ps.tile([128, 512], F32, tag="e", bufs=2)
                for par in (0, 1):
                    c = 2 * k + par
                    sl = slice(512 * c, 512 * (c + 1))
                    rsl = slice(U * par, U * par + U)
                    tp = (0, U * par)
                    nc.tensor.matmul(
                        e_ps[rsl], lhsT=w1s_sb[:, 0:U], rhs=st3[:, 0, sl],
                        start=True, stop=False, tile_position=tp,
                    )
                    nc.tensor.matmul(
                        e_ps[rsl], lhsT=w1s_sb[:, U : 2 * U], rhs=st3[:, 1, sl],
                        start=False, stop=True, tile_position=tp,
                    )
                nc.scalar.activation(
                    eT2s[b][:, 512 * k : 512 * (k + 1)], e_ps,
                    mybir.ActivationFunctionType.Tanh,
                    bias=cb_all[:, b : b + 1],
                )

            def logits_phase(b):
                # one [128,128] LDWEIGHTS + one 2-col matmul per 128 t's,
                # then exp with fused row-sums (on ACT, overlaps later e-MMs)
                eT2 = eT2s[b]
                lg = ps.tile([128, 2 * (T // 256)], F32, tag="lg", bufs=2)
                for j in range(T // 256):
                    nc.tensor.matmul(
                        lg[:, 2 * j : 2 * j + 2],
                        lhsT=eT2[:, 128 * j : 128 * (j + 1)],
                        rhs=w2t, start=True, stop=True,
                    )
                pAB = small_pool.tile([128, 2 * (T // 256)], BF16, tag="pAB")
                sums = small_pool.tile([128, 1], F32, tag="sums")
                nc.scalar.activation(
                    pAB, lg, mybir.ActivationFunctionType.Exp, accum_out=sums
                )
                pABs[b] = (pAB, sums)

            pool_state = [None] * B_CORE

            def z_phase(b):
                pAB, sums = pABs[b]
                z_ps = ps.tile([1, 1], F32, tag="warm", bufs=1)
                nc.tensor.matmul(z_ps, lhsT=sums, rhs=ones_col, start=True, stop=True)
                invz = small_pool.tile([1, 1], F32, tag="invz")
                nc.vector.reciprocal(invz, z_ps)
                pool_ps = ps.tile([1, D1], F32, tag="pool", bufs=1)
                pool_state[b] = (invz, pool_ps)

            def pool_half(b, half):
                pAB, sums = pABs[b]
                nat = nat_tiles[b]
                invz, pool_ps = pool_state[b]
                # pooling: tile n's weights live in pAB column col(n)
                for n in range(NT // 2 * half, NT // 2 * (half + 1)):
                    a_, r = n // 8, n % 8
                    col = 2 * (4 * a_ + r) if r < 4 else 2 * (4 * a_ + r - 4) + 1
                    nc.tensor.matmul(
                        pool_ps,
                        lhsT=pAB[:, col : col + 1],
                        rhs=nat[:, 256 * n : 256 * (n + 1)],
                        start=(n == 0),
                        stop=(n == NT - 1),
                    )
                if half == 1:
                    nc.scalar.activation(
                        final_sb[0:1, D1 * b : D1 * (b + 1)], pool_ps,
                        mybir.ActivationFunctionType.Copy, scale=invz,
                    )
                    nc.sync.dma_start(
                        out=outp[0:1, D1 * b : D1 * (b + 1)],
                        in_=final_sb[0:1, D1 * b : D1 * (b + 1)],
                    )

            # Software pipeline: batch b-1's logits and pool halves slot
            # between batch b's e-chunks, so by the time the PE reaches each
            # stage its ACT inputs (tanh / exp) are long since ready — no
            # head-of-line stalls on the PE queue.
            for b in range(B_CORE):
                if b + 4 < B_CORE:
                    load_seqt(b + 4)
                    load_nat(b + 4)
                eT2s[b] = et_pool.tile([128, T // 2], BF16, tag="eT2", name=f"eT2_{b}")
                e_chunk(b, 0)
                e_chunk(b, 1)
                if b >= 1:
                    logits_phase(b - 1)
                e_chunk(b, 2)
                e_chunk(b, 3)
                if b >= 1:
                    z_phase(b - 1)
                    pool_half(b - 1, 0)
                    pool_half(b - 1, 1)
            logits_phase(B_CORE - 1)
            z_phase(B_CORE - 1)
            pool_half(B_CORE - 1, 0)
            pool_half(B_CORE - 1, 1)

    nc.compile()
    return nc


_NC_CACHE = []


def _get_program():
    if not _NC_CACHE:
        _NC_CACHE.append(build_program())
    return _NC_CACHE[0]


def make_in_maps(sequence, context, W1, W2):
    sequence = np.ascontiguousarray(sequence, dtype=np.float32)
    context = np.ascontiguousarray(context, dtype=np.float32)
    W1 = np.ascontiguousarray(W1, dtype=np.float32)
    W2 = np.ascontiguousarray(W2, dtype=np.float32)

    # wbf = [w1s | w2two]: w1s[q, h*U+u] = W1[128h+q, u]
    wbf = np.zeros((128, 2 * U + 2), dtype=np.float32)
    wbf[:, 0 : 2 * U] = W1[:D1].reshape(2, 128, U).transpose(1, 0, 2).reshape(128, 2 * U)
    wbf[0:U, 2 * U] = W2[:, 0]
    wbf[U:128, 2 * U + 1] = W2[:, 0]
    wbf = wbf.astype(ml_dtypes.bfloat16)

    in_maps = []
    for c in range(N_CORES):
        sl = slice(B_CORE * c, B_CORE * (c + 1))
        s = sequence[sl]                                   # [8, 4096, 256]
        natp = np.ascontiguousarray(
            s.reshape(B_CORE, NT, 128, D1).transpose(0, 2, 1, 3)
            .reshape(B_CORE, 128, NT * D1)
        ).astype(ml_dtypes.bfloat16)
        # seqt[b, q, h*T+t] = seq[b, t, 128h+q]
        st = s.transpose(0, 2, 1)                          # [8, 256, 4096]
        seqt = np.ascontiguousarray(
            st.reshape(B_CORE, 2, 128, T).transpose(0, 2, 1, 3)
            .reshape(B_CORE, 128, 2 * T)
        ).astype(SEQT_NP)
        wf32 = np.concatenate([W1[D1:], context[sl].T], axis=1)  # [128, 72]
        in_maps.append(
            {
                "natp": natp,
                "seqt": seqt,
                "wbf": wbf,
                "wf32": np.ascontiguousarray(wf32),
            }
        )
    return in_maps


def kernel(sequence, context, W1, W2):
    """Full-input entry point: shards batch across 8 cores, returns [64, 256] f32."""
    from concourse.bass_utils import run_bass_kernel_spmd

    nc = _get_program()
    in_maps = make_in_maps(sequence, context, W1, W2)
    res = run_bass_kernel_spmd(nc, in_maps, list(range(N_CORES)))
    out = np.concatenate(
        [res.results[c]["outp"].reshape(B_CORE, D1) for c in range(N_CORES)], axis=0
    )
    return out.astype(np.float32)


# revision 27
# speedup vs baseline: 1.0647x; 1.0647x over previous
"""Context-aware attention pooling kernel for Trainium2 (8 NeuronCores).

Reference computation (per batch b):
    e      = tanh(seq @ W1[:256] + ctx @ W1[256:])      # [T, 64]
    logits = e @ W2                                      # [T, 1]
    a      = softmax(logits over T)
    out    = sum_t a[t] * seq[t]                         # [256]

Shapes: B=64, T=4096, D1=256, D2=128, UNITS=64.
Sharding: data-parallel over batch, 8 batches per core; W1/W2 replicated.

Host-side prep (make_in_maps) ships two copies of seq per core:
  - natp  [8, 128, 32*256] bf16: nat[b, p, n*256+d] = seq[b, 128n+p, d]
    (pool operand; t on partitions)
  - seqt  [8, 128, 2*4096] fp8e4m3: seqt[b, q, h*T+t] = seq[b, t, 128h+q]
    (e-matmul moving operand; d on partitions, pre-transposed on host so
    the PE does zero transposes; fp8 feeds only the tanh argument, the
    value path stays bf16)

Per-core program (per batch):
  - e-matmul: 4 PSUM tiles [128, 512]; even 512-chunk units on partitions
    0:64, odd on 64:128 (tile_position col split), K=256 via 2 accumulating
    matmuls vs the two seqt d-halves; tanh + ctx-bias fused on ScalarE in
    one [128, 512] activation per double-chunk
  - logits: one LDWEIGHTS (eT2 128x128 window) + one 2-column matmul per
    128 t's; rhs = [w2;0 | 0;w2] so even/odd chunk logits come out in one
    instruction pair; FWL stays enabled (no fp32 matmuls in steady state)
  - softmax without max-subtraction; single Exp over [128, 32] with fused
    row-sums; Z via ones-matmul; 1/Z applied to the pooled output
  - pooling on PE: p-columns stationary (1-col weight loads), nat tiles
    moving, accumulated over 32 t-tiles into PSUM [1, 256]
  - bf16 dummy matmuls trip the HAM clock gate during the DMA ramp
"""

import numpy as np
import ml_dtypes

import concourse.bacc as bacc
import concourse.mybir as mybir
from concourse.tile import TileContext

F32 = mybir.dt.float32
BF16 = mybir.dt.bfloat16
F8 = mybir.dt.float8e4

N_CORES = 8
B_CORE = 8          # batches per core
T = 4096
D1 = 256
D2 = 128
U = 64
NT = T // 128       # 32 t-tiles per batch

SEQT_FP8 = True     # fp8 e-path (rel err ~1.1e-2) vs bf16 (~2.5e-3)
SEQT_DT = F8 if SEQT_FP8 else BF16
SEQT_NP = ml_dtypes.float8_e4m3fn if SEQT_FP8 else ml_dtypes.bfloat16


def build_program():
    nc = bacc.Bacc("TRN2", target_bir_lowering=False, debug=False)

    natp = nc.declare_dram_parameter("natp", [B_CORE, 128, NT * D1], BF16, isOutput=False)
    seqt = nc.declare_dram_parameter("seqt", [B_CORE, 128, 2 * T], SEQT_DT, isOutput=False)
    # packed weights: one bf16 tensor (w1s | w2two) and one f32 (w1c | ctxT)
    # so the boot path is 2 SWDGE DMAs instead of 4
    wbf = nc.declare_dram_parameter("wbf", [128, 2 * U + 2], BF16, isOutput=False)
    wf32 = nc.declare_dram_parameter("wf32", [D2, U + B_CORE], F32, isOutput=False)
    outp = nc.declare_dram_parameter("outp", [1, B_CORE * D1], F32, isOutput=True)

    with TileContext(nc) as tc:
        with (
            tc.tile_pool(name="singles", bufs=1) as singles,
            tc.tile_pool(name="nat_pool", bufs=4) as nat_pool,
            tc.tile_pool(name="seqt_pool", bufs=4) as seqt_pool,
            tc.tile_pool(name="et_pool", bufs=2) as et_pool,
            tc.tile_pool(name="small_pool", bufs=2) as small_pool,
            tc.tile_pool(name="ps", bufs=1, space="PSUM") as ps,
        ):
            # Everything rides the gpsimd SWDGE queue: HWDGE descriptor
            # generation can't keep up with 128-partition APs (measured
            # ~20x starvation vs SWDGE), and a single FIFO queue delivers
            # bytes exactly in consumption order. Weights first (52 KB).
            wbf_sb = singles.tile([128, 2 * U + 2], BF16)
            nc.gpsimd.dma_start(out=wbf_sb, in_=wbf[:, :])
            wf32_sb = singles.tile([D2, U + B_CORE], F32)
            nc.gpsimd.dma_start(out=wf32_sb, in_=wf32[:, :])
            w1s_sb = wbf_sb[:, 0 : 2 * U]
            w2t = wbf_sb[:, 2 * U : 2 * U + 2]
            w1c_sb = wf32_sb[:, 0:U]
            ctxT_sb = wf32_sb[:, U : U + B_CORE]

            # per-batch seq loads, all on the same SWDGE queue in
            # consumption order: seqt (gates e-matmuls) then nat (pool)
            seqt_tiles = [None] * B_CORE
            nat_tiles = [None] * B_CORE

            def load_seqt(b, nchunks=2):
                st = seqt_pool.tile([128, 2 * T], SEQT_DT, tag="seqt", name=f"st{b}")
                st3 = st.rearrange("q (h t) -> q h t", h=2)
                src = seqt[b].rearrange("q (h t) -> q h t", h=2)
                for q in range(nchunks):
                    tsl = slice(T // nchunks * q, T // nchunks * (q + 1))
                    nc.gpsimd.dma_start(out=st3[:, :, tsl], in_=src[:, :, tsl])
                seqt_tiles[b] = st

            def load_nat(b):
                nat = nat_pool.tile([128, NT * D1], BF16, tag="nat", name=f"nat{b}")
                for q in range(4):
                    csl = slice(2048 * q, 2048 * (q + 1))
                    nc.gpsimd.dma_start(out=nat[:, csl], in_=natp[b][:, csl])
                nat_tiles[b] = nat

            ones_col = singles.tile([128, 1], F32)
            nc.vector.memset(ones_col, 1.0)

            load_seqt(0)
            load_nat(0)
            load_seqt(1)
            load_nat(1)
            load_seqt(2)
            load_nat(2)
            load_seqt(3)
            load_nat(3)

            # all 8 context projections, duplicated on both partition halves
            cb_ps = ps.tile([128, B_CORE], F32, tag="lg", bufs=2)
            nc.tensor.matmul(cb_ps[0:U], lhsT=w1c_sb, rhs=ctxT_sb, start=True, stop=True)
            nc.tensor.matmul(
                cb_ps[U:128], lhsT=w1c_sb, rhs=ctxT_sb, start=True, stop=True,
                tile_position=(0, U),
            )
            cb_all = singles.tile([128, B_CORE], F32)
            nc.scalar.copy(cb_all, cb_ps)

            # HAM warm-up: bf16 dummy matmuls during the DMA ramp so batch 0
            # computes at the full 2.4 GHz clock
            warm_ps = ps.tile([128, 128], F32, tag="warm", bufs=1)
            for _ in range(36):
                nc.tensor.matmul(warm_ps, lhsT=w1s_sb, rhs=w1s_sb, start=True, stop=True)

            final_sb = singles.tile([1, B_CORE * D1], F32)

            eT2s = [None] * B_CORE
            pABs = [None] * B_CORE

            def e_chunk(b, k):
                # eT2[:, 512k + i]: rows 0:64 = units for t = 1024k + i,
                # rows 64:128 = units for t = 1024k + 512 + i
                st3 = seqt_tiles[b].rearrange("q (h t) -> q h t", h=2)
                e_ps = ps.tile([128, 512], F32, tag="e", bufs=2)
                for par in (0, 1):
                    c = 2 * k + par
                    sl = slice(512 * c, 512 * (c + 1))
                    rsl = slice(U * par, U * par + U)
                    tp = (0, U * par)
                    nc.tensor.matmul(
                        e_ps[rsl], lhsT=w1s_sb[:, 0:U], rhs=st3[:, 0, sl],
                        start=True, stop=False, tile_position=tp,
                    )
                    nc.tensor.matmul(
                        e_ps[rsl], lhsT=w1s_sb[:, U : 2 * U], rhs=st3[:, 1, sl],
                        start=False, stop=True, tile_position=tp,
                    )
                nc.scalar.activation(
                    eT2s[b][:, 512 * k : 512 * (k + 1)], e_ps,
                    mybir.ActivationFunctionType.Tanh,
                    bias=cb_all[:, b : b + 1],
                )

            def logits_phase(b):
                # one [128,128] LDWEIGHTS + one 2-col matmul per 128 t's,
                # then exp with fused row-sums (on ACT, overlaps later e-MMs)
                eT2 = eT2s[b]
                lg = ps.tile([128, 2 * (T // 256)], F32, tag="lg", bufs=2)
                for j in range(T // 256):
                    nc.tensor.matmul(
                        lg[:, 2 * j : 2 * j + 2],
                        lhsT=eT2[:, 128 * j : 128 * (j + 1)],
                        rhs=w2t, start=True, stop=True,
                    )
                pAB = small_pool.tile([128, 2 * (T // 256)], BF16, tag="pAB")
                sums = small_pool.tile([128, 1], F32, tag="sums")
                nc.scalar.activation(
                    pAB, lg, mybir.ActivationFunctionType.Exp, accum_out=sums
                )
                pABs[b] = (pAB, sums)

            pool_state = [None] * B_CORE

            def z_phase(b):
                pAB, sums = pABs[b]
                z_ps = ps.tile([1, 1], F32, tag="warm", bufs=1)
                nc.tensor.matmul(z_ps, lhsT=sums, rhs=ones_col, start=True, stop=True)
                invz = small_pool.tile([1, 1], F32, tag="invz")
                nc.vector.reciprocal(invz, z_ps)
                pool_ps = ps.tile([1, D1], F32, tag="pool", bufs=1)
                pool_state[b] = (invz, pool_ps)

            def pool_half(b, half):
                pAB, sums = pABs[b]
                nat = nat_tiles[b]
                invz, pool_ps = pool_state[b]
                # pooling: tile n's weights live in pAB column col(n)
                for n in range(NT // 2 * half, NT // 2 * (half + 1)):
                    a_, r = n // 8, n % 8
                    col = 2 * (4 * a_ + r) if r < 4 else 2 * (4 * a_ + r - 4) + 1
                    nc.tensor.matmul(
                        pool_ps,
                        lhsT=pAB[:, col : col + 1],
                        rhs=nat[:, 256 * n : 256 * (n + 1)],
                        start=(n == 0),
                        stop=(n == NT - 1),
                    )
                if half == 1:
                    nc.scalar.activation(
                        final_sb[0:1, D1 * b : D1 * (b + 1)], pool_ps,
                        mybir.ActivationFunctionType.Copy, scale=invz,
                    )
                    nc.sync.dma_start(
                        out=outp[0:1, D1 * b : D1 * (b + 1)],
                        in_=final_sb[0:1, D1 * b : D1 * (b + 1)],
                    )

            # Software pipeline: batch b-1's logits and pool halves slot
            # between batch b's e-chunks, so by the time the PE reaches each
            # stage its ACT inputs (tanh / exp) are long since ready — no
            # head-of-line stalls on the PE queue.
            for b in range(B_CORE):
                if b + 4 < B_CORE:
                    load_seqt(b + 4)
                    load_nat(b + 4)
                eT2s[b] = et_pool.tile([128, T // 2], BF16, tag="eT2", name=f"eT2_{b}")
                e_chunk(b, 0)
                e_chunk(b, 1)
                if b >= 1:
                    logits_phase(b - 1)
                e_chunk(b, 2)
                e_chunk(b, 3)
                if b >= 1:
                    z_phase(b - 1)
                    pool_half(b - 1, 0)
                    pool_half(b - 1, 1)
            logits_phase(B_CORE - 1)
            z_phase(B_CORE - 1)
            pool_half(B_CORE - 1, 0)
            pool_half(B_CORE - 1, 1)

    nc.compile()
    return nc


_NC_CACHE = []


def _get_program():
    if not _NC_CACHE:
        _NC_CACHE.append(build_program())
    return _NC_CACHE[0]


def make_in_maps(sequence, context, W1, W2):
    sequence = np.ascontiguousarray(sequence, dtype=np.float32)
    context = np.ascontiguousarray(context, dtype=np.float32)
    W1 = np.ascontiguousarray(W1, dtype=np.float32)
    W2 = np.ascontiguousarray(W2, dtype=np.float32)

    # wbf = [w1s | w2two]: w1s[q, h*U+u] = W1[128h+q, u]
    wbf = np.zeros((128, 2 * U + 2), dtype=np.float32)
    wbf[:, 0 : 2 * U] = W1[:D1].reshape(2, 128, U).transpose(1, 0, 2).reshape(128, 2 * U)
    wbf[0:U, 2 * U] = W2[:, 0]
    wbf[U:128, 2 * U + 1] = W2[:, 0]
    wbf = wbf.astype(ml_dtypes.bfloat16)

    in_maps = []
    for c in range(N_CORES):
        sl = slice(B_CORE * c, B_CORE * (c + 1))
        s = sequence[sl]                                   # [8, 4096, 256]
        natp = np.ascontiguousarray(
            s.reshape(B_CORE, NT, 128, D1).transpose(0, 2, 1, 3)
            .reshape(B_CORE, 128, NT * D1)
        ).astype(ml_dtypes.bfloat16)
        # seqt[b, q, h*T+t] = seq[b, t, 128h+q]
        st = s.transpose(0, 2, 1)                          # [8, 256, 4096]
        seqt = np.ascontiguousarray(
            st.reshape(B_CORE, 2, 128, T).transpose(0, 2, 1, 3)
            .reshape(B_CORE, 128, 2 * T)
        ).astype(SEQT_NP)
        wf32 = np.concatenate([W1[D1:], context[sl].T], axis=1)  # [128, 72]
        in_maps.append(
            {
                "natp": natp,
                "seqt": seqt,
                "wbf": wbf,
                "wf32": np.ascontiguousarray(wf32),
            }
        )
    return in_maps


def kernel(sequence, context, W1, W2):
    """Full-input entry point: shards batch across 8 cores, returns [64, 256] f32."""
    from concourse.bass_utils import run_bass_kernel_spmd

    nc = _get_program()
    in_maps = make_in_maps(sequence, context, W1, W2)
    res = run_bass_kernel_spmd(nc, in_maps, list(range(N_CORES)))
    out = np.concatenate(
        [res.results[c]["outp"].reshape(B_CORE, D1) for c in range(N_CORES)], axis=0
    )
    return out.astype(np.float32)
